# revision 6
# baseline (speedup 1.0000x reference)
"""Trainium2 Bass kernel for nn_CDF_quadratic (piecewise-quadratic CDF flow).

Strategy (no per-element gather needed):
  - The CDF y(x) restricted to region h = k//4 (16 regions of 4 bins) equals a
    recentered quadratic (all knots below the region folded in, f64 on host)
    plus <=3 residual relu^2 knot terms inside the region.
  - Per-element region parameters (10 per (element-column)) are fetched
    EXACTLY on the TensorEngine: params_c(h) = C + sum_d (dP_d/2)*sign(tau -
    4d + 32), a 16-row sign-basis contraction with a block-diagonal
    per-column parameter table. sign basis is computed by one ACT Sign pass
    over a PE-replicated continuous bin coordinate tau (no floor needed).
  - tau = sign(x)*log1p(|x|(R-1)/x1L)/ln(R) analytically inverts the
    geometric mesh (one ACT Ln pass).
Sharding: pure data parallel over samples across 8 NeuronCores.
"""
import sys

sys.path.insert(0, "/opt/trn_rl_repo")

import numpy as np

N_BINS = 64
D = 8
RR = 1.2
BOUND = 10.0
BETA = 1e-06
N_TOTAL = 2097152
N_CORES = 8
S_SHARD = N_TOTAL // N_CORES          # 262144 samples per core
P = 128
F = 512                               # elements per partition per tile
TILES = S_SHARD * D // (P * F)        # 32
NREG = 16
NPAR = 10                             # a, b, c, X, k1, m1, k2, m2, k3, m3


# ----------------------------------------------------------------------------
# host-side table construction (f64)
# ----------------------------------------------------------------------------
def _host_tables(p):
    m = N_BINS / 2
    idx = np.arange(N_BINS + 1, dtype=np.float64) - m
    x1L = BOUND * (RR - 1.0) / (RR ** m - 1.0)
    xr = (1.0 - RR ** np.abs(idx)) / (1.0 - RR)
    xr = np.where(idx >= 0, x1L * xr, -x1L * xr)
    xr = (xr + BOUND) / (2.0 * BOUND)
    mesh = np.concatenate([[0.0], xr[1:-1], [1.0]])
    elmt = mesh[1:] - mesh[:-1]

    p = p.astype(np.float64)
    p0 = np.full((1, D), BETA)
    ep = np.exp(p)
    px = ep * ((elmt[:-1] + elmt[1:]) / 2.0)[:, None]
    scale = (1.0 - (elmt[0] + elmt[-1]) * BETA / 2.0) / np.sum(px, 0, keepdims=True)
    px = scale * ep
    pdf = np.concatenate([p0, px, p0], 0)            # [65, D]
    cell = (pdf[:-1] + pdf[1:]) / 2.0 * elmt[:, None]
    Fref = np.concatenate([np.zeros((1, D)), np.cumsum(cell, 0)[:N_BINS - 1]], 0)

    m_t = 20.0 * mesh - 10.0                          # x-space mesh [65]
    s_n = (pdf[1:] - pdf[:-1]) / elmt[:, None]        # [64, D]
    A = 20.0 * Fref - 10.0                            # y at bin left edge
    V = pdf[:N_BINS].copy()                           # dld at bin left edge
    C = s_n / 40.0                                    # x-space quad coeff

    # per-region recentered params
    Xh = np.zeros(NREG)
    par = np.zeros((NREG, D, NPAR))
    for h in range(NREG):
        k0 = 4 * h
        X = m_t[k0]
        Xh[h] = X
        par[h, :, 0] = A[k0]
        par[h, :, 1] = V[k0]
        par[h, :, 2] = C[k0]
        par[h, :, 3] = X
        for r in range(3):
            j = k0 + 1 + r
            par[h, :, 4 + 2 * r] = C[j] - C[j - 1]
            par[h, :, 5 + 2 * r] = m_t[j] - X
    x1L_x = x1L  # x-space first-cell width
    return par, x1L_x


def _const_arrays(p):
    """All device constant tensors."""
    par, x1L_x = _host_tables(p)       # [16, 8, 10]

    ident = np.eye(P, dtype=np.float32)

    # SEL matrices: SEL[j][i, m=(c*16+d)] = 1 if i == 8j + c
    sel = np.zeros((16, P, P), np.float32)
    for j in range(16):
        for c in range(D):
            for d in range(NREG):
                sel[j, 8 * j + c, c * 16 + d] = 1.0

    # sign bias per partition row (c*16 + d): d=0 -> always +1; else 32 - 4d
    bias = np.zeros((P, 1), np.float32)
    for c in range(D):
        for d in range(NREG):
            bias[c * 16 + d, 0] = 1000.0 if d == 0 else np.float32(32.0 - 4.0 * d)

    # PT [(c',16d), (c,t)] block-diag sign-basis coefficients
    PT = np.zeros((P, D * NPAR), np.float32)
    for c in range(D):
        Pv = par[:, c, :]                      # [16, NPAR] f64
        Cp = 0.5 * (Pv[0] + Pv[15])            # [NPAR]
        dP = 0.5 * (Pv[1:] - Pv[:-1])          # [15, NPAR]
        # device basis v_d = 2*max(sign,0) in {0,2} (sign==0 -> v=0 ->
        # lower region, exact by C^1 continuity); algebra:
        # G = Cp + sum dP_d*s_d = (v_0/2)*(Cp - sum dP_d) + sum dP_d*v_d
        for t in range(NPAR):
            PT[c * 16 + 0, c * NPAR + t] = 0.5 * (Cp[t] - dP[:, t].sum())
            for d in range(1, 16):
                PT[c * 16 + d, c * NPAR + t] = dP[d - 1, t]

    cc = np.float32((RR - 1.0) / x1L_x)
    inv_lnr = np.float32(1.0 / np.log(RR))
    return dict(ident=ident, sel=sel.transpose(1, 0, 2).reshape(P, 16 * P),
                bias=bias, PT=PT), cc, inv_lnr


# ----------------------------------------------------------------------------
# device program
# ----------------------------------------------------------------------------
_CACHE = {}


def _build_nc(consts, cc, inv_lnr, s_shard=S_SHARD):
    import concourse.bacc as bacc
    import concourse.mybir as mybir
    from concourse.tile import TileContext

    f32 = mybir.dt.float32
    AF = mybir.ActivationFunctionType
    OP = mybir.AluOpType

    nc = bacc.Bacc("TRN2", target_bir_lowering=False, debug=False,
                   num_devices=N_CORES)
    tiles = s_shard * D // (P * F)
    x_in = nc.dram_tensor("x", [s_shard, D], f32, kind="ExternalInput")
    ld_in = nc.dram_tensor("logdet", [s_shard, 1], f32, kind="ExternalInput")
    ident_in = nc.dram_tensor("ident", [P, P], f32, kind="ExternalInput")
    sel_in = nc.dram_tensor("sel", [P, 16 * P], f32, kind="ExternalInput")
    bias_in = nc.dram_tensor("bias", [P, 1], f32, kind="ExternalInput")
    pt_in = nc.dram_tensor("pt", [P, D * NPAR], f32, kind="ExternalInput")
    y_out = nc.dram_tensor("y", [s_shard, D], f32, kind="ExternalOutput")
    ldo = nc.dram_tensor("ld", [s_shard, 1], f32, kind="ExternalOutput")

    xv = x_in.ap().rearrange("(p a) d -> p (a d)", p=P)       # [128, 16384]
    yv = y_out.ap().rearrange("(p a) d -> p (a d)", p=P)
    ldi_v = ld_in.ap().rearrange("(p a) o -> p (a o)", p=P)   # [128, 2048]
    ldo_v = ldo.ap().rearrange("(p a) o -> p (a o)", p=P)

    NB = F // P                                               # 4 x_T blocks
    SPT = F // D                                              # samples per tile/part

    with TileContext(nc) as tc:
        with (
            tc.tile_pool(name="const", bufs=1) as cpool,
            tc.tile_pool(name="io", bufs=3) as iop,
            tc.tile_pool(name="work", bufs=2) as wp,
            tc.tile_pool(name="sgn", bufs=2) as sp,
            tc.tile_pool(name="gp", bufs=2) as gpool,
            tc.tile_pool(name="ps", bufs=2, space="PSUM") as psp,
            tc.tile_pool(name="psg", bufs=2, space="PSUM") as psg,
            tc.tile_pool(name="pst", bufs=2, space="PSUM") as pst,
        ):
            ident = cpool.tile([P, P], f32, tag="ident")
            sel = cpool.tile([P, 16 * P], f32, tag="sel")
            sbias = cpool.tile([P, 1], f32, tag="bias")
            pt = cpool.tile([P, D * NPAR], f32, tag="pt")
            nc.sync.dma_start(ident[:], ident_in.ap())
            nc.sync.dma_start(sel[:], sel_in.ap())
            nc.sync.dma_start(sbias[:], bias_in.ap())
            nc.sync.dma_start(pt[:], pt_in.ap())

            for ti in range(tiles):
                xs = slice(ti * F, (ti + 1) * F)
                x_t = iop.tile([P, F], f32, tag="x")
                nc.sync.dma_start(x_t[:], xv[:, xs])

                # ---- transpose x -> x_T ----
                x_T = wp.tile([P, F], f32, tag="xT")
                for b in range(NB):
                    ptile = pst.tile([P, P], f32, tag="tp")
                    nc.tensor.transpose(ptile[:], x_t[:, b * P:(b + 1) * P],
                                        ident[:])
                    nc.vector.tensor_copy(x_T[:, b * P:(b + 1) * P], ptile[:])

                # ---- tau on x_T ----
                ax = wp.tile([P, F], f32, tag="ax")
                nc.scalar.activation(ax[:], x_T[:], AF.Abs)
                g_t = wp.tile([P, F], f32, tag="g")
                nc.scalar.activation(g_t[:], ax[:], AF.Ln, bias=1.0,
                                     scale=float(cc))
                sg_t = wp.tile([P, F], f32, tag="sg")
                nc.scalar.activation(sg_t[:], x_T[:], AF.Sign)
                tau = wp.tile([P, F], f32, tag="tau")
                nc.vector.scalar_tensor_tensor(tau[:], g_t[:], float(inv_lnr),
                                               sg_t[:], OP.mult, OP.mult)

                # ---- per j: replicate, sign, fetch ----
                G = gpool.tile([P, NB * 16 * D * NPAR], f32, tag="G")  # 5120
                W = D * NPAR
                gv4 = G[:].rearrange("p (b jj w) -> p jj b w", b=NB, jj=16,
                                     w=W)
                for j in range(16):
                    rep = psp.tile([P, F], f32, tag="rep")
                    nc.tensor.matmul(rep[:], sel[:, j * P:(j + 1) * P], tau[:])
                    sgn_j = sp.tile([P, F], f32, tag="sgnj")
                    nc.scalar.activation(sgn_j[:], rep[:], AF.Sign,
                                         bias=sbias[:, 0:1])
                    nc.vector.tensor_scalar(sgn_j[:], sgn_j[:], 0.0, 2.0,
                                            OP.max, OP.mult)
                    gps = psg.tile([P, NB * W], f32, tag="gps")
                    for b in range(NB):
                        nc.tensor.matmul(gps[:, b * W:(b + 1) * W],
                                         sgn_j[:, b * P:(b + 1) * P], pt[:])
                    nc.scalar.activation(
                        gv4[:, j],
                        gps[:].rearrange("p (b w) -> p b w", b=NB), AF.Copy)

                # ---- finals in natural frame ----
                # param view: t-th param of element i at G[p, i*NPAR + t]
                gvw = G[:].rearrange("p (i t) -> p t i", t=NPAR)

                def pv(t):
                    return gvw[:, t]

                xp = wp.tile([P, F], f32, tag="xp")       # x' = x - X
                nc.vector.tensor_tensor(xp[:], x_t[:], pv(3), OP.subtract)
                m1 = wp.tile([P, F], f32, tag="m1")
                nc.vector.tensor_tensor(m1[:], xp[:], pv(2), OP.mult)
                a1 = wp.tile([P, F], f32, tag="a1")
                nc.vector.tensor_tensor(a1[:], m1[:], pv(1), OP.add)
                yb = wp.tile([P, F], f32, tag="yb")
                nc.vector.tensor_tensor(yb[:], xp[:], a1[:], OP.mult)
                nc.vector.tensor_tensor(yb[:], yb[:], pv(0), OP.add)
                db = wp.tile([P, F], f32, tag="db")
                nc.vector.scalar_tensor_tensor(db[:], m1[:], 2.0, pv(1),
                                               OP.mult, OP.add)
                for r in range(3):
                    u_r = wp.tile([P, F], f32, tag="ur")
                    nc.vector.tensor_tensor(u_r[:], xp[:], pv(5 + 2 * r),
                                            OP.subtract)
                    rr = wp.tile([P, F], f32, tag="rr")
                    nc.scalar.activation(rr[:], u_r[:], AF.Relu)
                    t3 = wp.tile([P, F], f32, tag="t3")
                    nc.vector.tensor_tensor(t3[:], rr[:], pv(4 + 2 * r),
                                            OP.mult)
                    nc.vector.scalar_tensor_tensor(db[:], t3[:], 2.0, db[:],
                                                   OP.mult, OP.add)
                    nc.vector.tensor_tensor(t3[:], t3[:], rr[:], OP.mult)
                    nc.vector.tensor_tensor(yb[:], yb[:], t3[:], OP.add)

                nc.sync.dma_start(yv[:, xs], yb[:])

                lg = wp.tile([P, F], f32, tag="lg")
                nc.scalar.activation(lg[:], db[:], AF.Ln)
                lred = wp.tile([P, SPT], f32, tag="lred")
                nc.vector.tensor_reduce(
                    lred[:], lg[:].rearrange("p (s c) -> p s c", c=D),
                    mybir.AxisListType.X, OP.add)
                ldt = iop.tile([P, SPT], f32, tag="ldt")
                nc.sync.dma_start(ldt[:], ldi_v[:, ti * SPT:(ti + 1) * SPT])
                nc.vector.tensor_tensor(lred[:], lred[:], ldt[:], OP.add)
                nc.sync.dma_start(ldo_v[:, ti * SPT:(ti + 1) * SPT], lred[:])

    nc.compile()
    return nc


# ----------------------------------------------------------------------------
# entry point
# ----------------------------------------------------------------------------
def kernel(x, logdet, p):
    from concourse.bass_utils import run_bass_kernel_spmd

    consts, cc, inv_lnr = _const_arrays(np.asarray(p))
    key = "nc"
    if key not in _CACHE:
        _CACHE[key] = _build_nc(consts, cc, inv_lnr)
    nc = _CACHE[key]

    x = np.ascontiguousarray(np.asarray(x, np.float32))
    logdet = np.ascontiguousarray(np.asarray(logdet, np.float32))
    in_maps = []
    for core in range(N_CORES):
        s = slice(core * S_SHARD, (core + 1) * S_SHARD)
        in_maps.append({
            "x": x[s], "logdet": logdet[s],
            "ident": consts["ident"], "sel": consts["sel"],
            "bias": consts["bias"], "pt": consts["PT"],
        })
    res = run_bass_kernel_spmd(nc, in_maps, core_ids=list(range(N_CORES)))
    y = np.concatenate([r["y"] for r in res.results], 0)
    ld = np.concatenate([r["ld"] for r in res.results], 0)
    return y, ld
